# revision 19
# baseline (speedup 1.0000x reference)
"""Trainium2 Bass kernel for nn_AttentionalGNN_81982335746601.

Computation (reference semantics, full shapes):
  desc0 (1,128,128), desc1 (1,128,2048), dist (1,128,128,2048)
  layer0: desc{0,1} += AttentionalPropagation_self(desc{0,1})  [shared weights]
  layer1: out = MLP([3D,D,D]) over per-pair concat(q_i, k_j, dist_ij)
          -> (128, 2048, 128), softmax-free.

Sharding: core p takes query rows i in [256p, 256p+256).  Its dist slice
dist[0, 16p:16p+16, :, :] is exactly the dist_flat columns it needs, and the
layer-0 self-attention over desc1 is sharded over the same query rows, so no
cross-core communication is needed.  desc0's branch and desc1's K/V are
computed replicated on every core.

All data is bf16 (PSUM accumulation stays f32); measured end-to-end max rel
err ~4e-3 vs the f32 reference.  Engine balancing:
  - attention softmax denominator comes from a 33rd all-ones column appended
    to each head's V block (VTE), so no separate ones-matmul row-sum;
    1/r is broadcast to 32 partitions with a rank-1 matmul.
  - conv biases ride the PSUM->SBUF copies (ACT bias / Pool tensor_scalar),
    a_bv is folded into a_bm host-side (post-softmax-mean bias commutes
    through the wm conv).
  - cross-MLP per 512-pair chunk: hp = cwd@dist (+ cwk@k via PE matmul or
    DVE add, per-chunk class), relu with per-q-group bias (cwq@q + cb1)
    as 4x128 tensor_scalar/activation on Pool/ACT, out = cw2@hr, then
    +cb2 and bf16 conversion on ACT/DVE/Pool per-chunk class.
  - dist loads and out stores are 1MB DMAs on the SP queue.
"""

import numpy as np
import ml_dtypes
from contextlib import ExitStack

import concourse.bacc as bacc
import concourse.mybir as mybir
from concourse.tile import TileContext
from concourse.bass_utils import run_bass_kernel_spmd

F32 = mybir.dt.float32
BF16 = mybir.dt.bfloat16
AF = mybir.ActivationFunctionType
ALU = mybir.AluOpType

D = 128
H = 4
DH = 32
N0 = 128
N1 = 2048
NCORES = 8
NQL = N1 // NCORES            # 256 local query nodes
NDSL = N0 // NCORES           # 16 dist d-slices per core
NCH = 4 * NDSL                # 64 phase-B chunks of 512 pair-columns
SCALE = float(1.0 / np.sqrt(DH))

# weight blocks packed into BIGB, in order (all [128,128] unless noted)
_WNAMES = ["wkT", "wqT0", "wqT1", "wqT2", "wqT3", "wvT", "wmT",
           "w1T00", "w1T10", "w1T01", "w1T11", "w2T0", "w2T1",
           "cwq", "cwk", "cwd", "cw2"]
BIGB_COLS = len(_WNAMES) * D + D + NQL + DH   # weights + d0 + d1loc + ones32
NBIAS = 16                                     # BIGF bias columns

# phase B runs in 32 pairs of 512-col chunks (1024-wide PSUM tiles).
# Pair routes (tunable balance knobs):
#   PE pairs: cwk/cwq delivered as PE matmuls, one 1024-wide ACT relu
#   default:  DVE adds CK (tiled) into SBUF, Pool does relu with per-q
#             (cwq@q + cb1) bias columns
# out-drain: ACT with cb2 bias, except OD_DVE pairs on DVE.
NPAIR = NCH // 2
EARLY_P = 20                                  # pairs whose cwd+ck front-end
                                              # runs during phase A
PAIR_PE = frozenset((20, 22, 24, 26, 28, 30))  # cwk/cwq as PE matmuls
OD_DVE = frozenset((1, 4, 7, 10, 13, 16, 19, 21, 23, 25))
RELU_ACT = frozenset()                        # post-A relu on ACT
RELU_DVE = frozenset((0, 2, 4, 6, 8, 10, 12, 14))  # relu on DVE (2x mode)

_CACHE: dict = {}


def _build(trace_sim: bool = False, debug_taps: bool = False):
    nc = bacc.Bacc("TRN2", target_bir_lowering=False, debug=False,
                   num_devices=NCORES)

    bigb = nc.dram_tensor("bigb", [D, BIGB_COLS], BF16,
                          kind="ExternalInput").ap()
    d1d = nc.dram_tensor("d1d", [D, N1], BF16, kind="ExternalInput").ap()
    bigf = nc.dram_tensor("bigf", [D, NBIAS], F32, kind="ExternalInput").ap()
    dist = nc.dram_tensor("dist", [NDSL, N0, N1], BF16,
                          kind="ExternalInput").ap()
    out = nc.dram_tensor("out", [D, NQL * N0], BF16,
                         kind="ExternalOutput").ap()

    with TileContext(nc, trace_sim=trace_sim) as tc:
        with ExitStack() as st:
            cp = st.enter_context(tc.tile_pool(name="consts", bufs=1))
            ap_ = st.enter_context(tc.tile_pool(name="apool", bufs=1))
            # phase-B input pool opened early so dist prefetch DMAs can be
            # hoisted to t=0 by the scheduler
            bip = st.enter_context(tc.tile_pool(name="bin", bufs=1))
            ehp = st.enter_context(tc.tile_pool(name="ehs", bufs=1))

            BIGB = cp.tile([D, BIGB_COLS], BF16, name="BIGB")
            nc.sync.dma_start(out=BIGB[:], in_=bigb[:])
            D1 = cp.tile([D, N1], BF16, name="D1")
            nc.sync.dma_start(out=D1[:], in_=d1d[:])
            BIGF = cp.tile([D, NBIAS], F32, name="BIGF")
            nc.sync.dma_start(out=BIGF[:], in_=bigf[:])

            W = {}
            for i, nm in enumerate(_WNAMES):
                W[nm] = BIGB[:, D * i:D * (i + 1)]
            nw = len(_WNAMES)
            D0 = BIGB[:, nw * D:(nw + 1) * D]
            D1L = BIGB[:, (nw + 1) * D:(nw + 1) * D + NQL]
            ONES32 = BIGB[0:1, (nw + 1) * D + NQL:(nw + 1) * D + NQL + DH]
            # f32 bias columns in BIGF
            BQC = [BIGF[:, h:h + 1] for h in range(H)]  # masked bq*scale
            BKC = BIGF[:, 4:5]
            BMPC = BIGF[:, 5:6]      # bm + wm @ bv
            B1T = BIGF[:, 6:7]
            B1B = BIGF[:, 7:8]
            B2C = BIGF[:, 8:9]
            CB1C = BIGF[:, 9:10]
            CB2C = BIGF[:, 10:11]

            OMS = {}
            stP = ExitStack()
            psPOH = stP.enter_context(tc.tile_pool(name="psPOH", bufs=1,
                                                   space="PSUM"))
            psA = stP.enter_context(tc.tile_pool(name="psA", bufs=1,
                                                 space="PSUM"))
            sm = stP.enter_context(tc.tile_pool(name="smlp", bufs=2))
            ptp = stP.enter_context(tc.tile_pool(name="ptp", bufs=3))

            def conv_stage(x_full, x_q, n_kv, n_q, tagn):
                """q/k/v convolutions, all bf16.

                K packed (128, n_kv); biases folded into the PSUM->SBUF
                copies.  Q produced 4x with per-head masked weights
                (host-side).  VTE packs per-(j,head) 32 V columns plus a
                33rd all-ones column (for the softmax denominator)."""
                nm = n_kv // 128
                K = ap_.tile([D, n_kv], BF16, name=f"K{tagn}")
                QH = []
                VTE = ap_.tile([D, nm * H * (DH + 1)], BF16,
                               name=f"VTE{tagn}")
                vv = VTE[:].rearrange("p (g c) -> p g c", c=DH + 1)
                nc.vector.memset(vv[:, :, DH:DH + 1], 1.0)
                for c0 in range(0, n_kv, 512):
                    w = min(512, n_kv - c0)
                    pk = psA.tile([D, 512], F32, name="pk",
                                  tag=f"pk{tagn}", bufs=2)[:, :w]
                    nc.tensor.matmul(pk, W["wkT"], x_full[:, c0:c0 + w],
                                     start=True, stop=True)
                    nc.scalar.activation(K[:, c0:c0 + w], pk,
                                         AF.Identity, bias=BKC)
                for h in range(H):
                    pq = psA.tile([D, 512], F32, name="pq",
                                  tag=f"pk{tagn}", bufs=2)[:, :n_q]
                    nc.tensor.matmul(pq, W[f"wqT{h}"], x_q, start=True,
                                     stop=True)
                    Qh = ap_.tile([D, 256], BF16,
                                  name=f"Q{tagn}{h}")[:, :n_q]
                    if tagn == "1":
                        nc.scalar.activation(Qh, pq, AF.Identity,
                                             bias=BQC[h])
                    else:
                        nc.vector.tensor_scalar(Qh, pq, BQC[h], None,
                                                op0=ALU.add)
                    QH.append(Qh)
                # VTE[m, (j,h,d)] = sum_c x[c,m] wvT[c,d]  (bv folded to bm')
                for j in range(nm):
                    pv = psA.tile([D, 128], F32, name="pv",
                                  tag=f"pk{tagn}", bufs=2)
                    nc.tensor.matmul(pv, x_full[:, 128 * j:128 * j + 128],
                                     W["wvT"], start=True, stop=True)
                    dst = vv[:, H * j:H * (j + 1), 0:DH]
                    src = pv[:].rearrange("p (h c) -> p h c", h=H)
                    if tagn == "1":
                        nc.scalar.copy(dst, src)
                    else:
                        nc.vector.tensor_copy(dst, src)
                return K, QH, VTE

            def prop(stage, x_q, n_kv, n_q, tagn):
                """Attention + MLP; returns x_q + MLP update (bf16)."""
                nm = n_kv // 128
                K, QH, VTE = stage
                vv = VTE[:].rearrange("p (g c) -> p g c", c=DH + 1)
                nsg = (H * n_q + 511) // 512        # 512-wide score groups
                hpg = 512 // n_q                    # heads per group
                # heads packed 2-per-PSUM-bank on the partition axis at the
                # legal matmul base partitions 0 and 64; row 32 (resp. 96)
                # is the softmax denominator from VTE's all-ones column.
                # d0 (n_q=128) packs all 4 heads into one bank (single-shot
                # accumulation groups, so sharing a bank is safe).
                if n_q <= 128:
                    POH0 = psPOH.tile([64 + DH + 1, 512], F32,
                                      name=f"poh{tagn}")

                    def poh(h):
                        return POH0[64 * (h % 2):64 * (h % 2) + DH + 1,
                                    n_q * (h // 2):n_q * (h // 2) + n_q]
                else:
                    POHP = [psPOH.tile([64 + DH + 1, 256], F32,
                                       name=f"poh{tagn}{t}")[:, :n_q]
                            for t in range(H // 2)]

                    def poh(h):
                        return POHP[h // 2][64 * (h % 2):
                                            64 * (h % 2) + DH + 1, :]

                for j in range(nm):
                    PTs = []
                    for gi in range(nsg):
                        psg = psA.tile([D, 512], F32, name=f"psg{gi}",
                                       tag=f"pk{tagn}", bufs=2)
                        for hh in range(hpg):
                            h = gi * hpg + hh
                            nc.tensor.matmul(
                                psg[:, hh * n_q:(hh + 1) * n_q],
                                K[:, 128 * j:128 * j + 128],
                                QH[h], start=True, stop=True)
                        PT = ptp.tile([D, 512], BF16, name="pt")
                        nc.scalar.activation(PT[:], psg[:], AF.Exp)
                        PTs.append(PT)
                    for h in range(H):
                        PT = PTs[h // hpg]
                        nc.tensor.matmul(
                            poh(h),
                            vv[:, H * j + h, :],
                            PT[:, (h % hpg) * n_q:(h % hpg + 1) * n_q],
                            start=(j == 0), stop=(j == nm - 1))
                # 1/r per (head, query), broadcast to 32 partitions via a
                # rank-1 matmul, then merge heads
                RIR = ap_.tile([1, H * 256], BF16, name=f"RIR{tagn}")[:,
                                                                      :H * n_q]
                with nc.allow_low_precision(
                        reason="bf16 softmax 1/r; validated ~4e-3 end-to-end"):
                    for h in range(H):
                        nc.vector.reciprocal(RIR[:, h * n_q:(h + 1) * n_q],
                                             poh(h)[DH:DH + 1, :])
                OM = ap_.tile([D, n_q], BF16, name=f"OM{tagn}")
                RI32S = ap_.tile([DH, H * 256], F32,
                                 name=f"ris{tagn}")[:, :H * n_q]
                hpr = 512 // n_q                 # heads per 512-wide bcast
                for h0 in range(0, H, hpr):
                    RI32 = psA.tile([DH, 512], F32, name=f"ri{tagn}",
                                    tag="ri", bufs=1)[:, :hpr * n_q]
                    nc.tensor.matmul(RI32, ONES32,
                                     RIR[:, h0 * n_q:(h0 + hpr) * n_q],
                                     start=True, stop=True)
                    nc.vector.tensor_copy(
                        RI32S[:, h0 * n_q:(h0 + hpr) * n_q], RI32)
                    for hh in range(hpr):
                        h = h0 + hh
                        nc.vector.tensor_mul(
                            OM[DH * h:DH * h + DH, :], poh(h)[0:DH, :],
                            RI32S[:, (h0 + hh) * n_q:(h0 + hh + 1) * n_q])
                OMS[tagn] = OM
                # msg + MLP epilogue
                pm = psA.tile([D, 256], F32, name="pm",
                              tag=f"pk{tagn}", bufs=2)[:, :n_q]
                nc.tensor.matmul(pm, W["wmT"], OM[:], start=True, stop=True)
                MSG = sm.tile([D, 256], BF16, name="msg")[:, :n_q]
                nc.scalar.activation(MSG, pm, AF.Identity, bias=BMPC)
                ph1 = psA.tile([D, 256], F32, name="pm",
                               tag=f"pk{tagn}", bufs=2)[:, :n_q]
                nc.tensor.matmul(ph1, W["w1T00"], x_q, start=True,
                                 stop=False)
                nc.tensor.matmul(ph1, W["w1T10"], MSG, start=False,
                                 stop=True)
                HT = sm.tile([D, 256], BF16, name="ht")[:, :n_q]
                nc.scalar.activation(HT, ph1, AF.Relu, bias=B1T)
                ph2 = psA.tile([D, 256], F32, name="pm",
                               tag=f"pk{tagn}", bufs=2)[:, :n_q]
                nc.tensor.matmul(ph2, W["w1T01"], x_q, start=True,
                                 stop=False)
                nc.tensor.matmul(ph2, W["w1T11"], MSG, start=False,
                                 stop=True)
                HB = sm.tile([D, 256], BF16, name="hb")[:, :n_q]
                nc.scalar.activation(HB, ph2, AF.Relu, bias=B1B)
                py = psA.tile([D, 256], F32, name="pm",
                              tag=f"pk{tagn}", bufs=2)[:, :n_q]
                nc.tensor.matmul(py, W["w2T0"], HT, start=True, stop=False)
                nc.tensor.matmul(py, W["w2T1"], HB, start=False, stop=True)
                DN = ap_.tile([D, n_q], BF16, name=f"DN{tagn}")
                nc.vector.scalar_tensor_tensor(DN[:], py, B2C, x_q,
                                               op0=ALU.add, op1=ALU.add)
                return DN

            st0 = conv_stage(D0, D0, N0, N0, "0")
            DN0 = prop(st0, D0, N0, N0, "0")

            # k-side phase-B contributions, available as soon as d0 is done
            KB = ap_.tile([D, 512], BF16, name="KB")
            nc.gpsimd.tensor_copy(
                KB[:].rearrange("p (a b) -> p a b", a=4),
                DN0[:].unsqueeze(1).broadcast_to([D, 4, 128]))
            pck = psA.tile([D, 128], F32, name="pck", tag="pk0", bufs=2)
            nc.tensor.matmul(pck, W["cwk"], DN0[:], start=True, stop=True)
            CKB1 = ap_.tile([D, 128], F32, name="CKB1")
            nc.scalar.copy(CKB1[:], pck)
            CKB8 = ap_.tile([D, 1024], F32, name="CKB8")
            nc.gpsimd.tensor_copy(
                CKB8[:].rearrange("p (a b) -> p a b", a=8),
                CKB1[:].unsqueeze(1).broadcast_to([D, 8, 128]))

            # dist loads for all 2-slice blocks (scheduler streams them)
            dints = {}
            for bb in range(NDSL // 2):
                dints[bb] = bip.tile([D, 2 * N1], BF16, name=f"di{bb}",
                                      tag="di", bufs=6)
                for a in range(2):
                    nc.sync.dma_start(
                        out=dints[bb][:, N1 * a:N1 * (a + 1)],
                        in_=dist[2 * bb + a])

            st1 = conv_stage(D1, D1L, N1, NQL, "1")
            DN1 = prop(st1, D1L, N1, NQL, "1")
            pcq = psA.tile([D, 256], F32, name="pcq", tag="pk1", bufs=2)
            nc.tensor.matmul(pcq, W["cwq"], DN1[:], start=True, stop=True)
            CQB1 = ap_.tile([D, NQL], F32, name="CQB1")
            nc.scalar.activation(CQB1[:], pcq, AF.Identity, bias=CB1C)

            # early phase-B front-ends: cwd matmul + CK add for the first
            # EARLY_P pairs, overlapped with the d1 branch (PE/DVE are
            # mostly idle during its ACT-bound softmax stream)
            HS = {}
            for u in range(EARLY_P):
                bb, s = divmod(u, 4)
                HS[u] = ehp.tile([D, 1024], BF16, name=f"ehs{u}",
                                  tag="ehs", bufs=EARLY_P)
                for hh in range(2):
                    hp5 = psA.tile([D, 512], F32, name="hp5", tag="pk0",
                                   bufs=2)
                    dsl = slice(512 * (2 * s + hh), 512 * (2 * s + hh) + 512)
                    nc.tensor.matmul(hp5[:], W["cwd"], dints[bb][:, dsl],
                                     start=True, stop=True)
                    nc.vector.tensor_tensor(
                        HS[u][:, 512 * hh:512 * hh + 512], hp5[:],
                        CKB8[:, 0:512], op=ALU.add)

            if debug_taps:
                for nm_, t_ in [("DN0", DN0), ("DN1", DN1),
                                ("CKB8", CKB8), ("CQB1", CQB1),
                                ("KB", KB)]:
                    dbg = nc.dram_tensor(f"dbg_{nm_}", list(t_.shape),
                                         t_.dtype,
                                         kind="ExternalOutput").ap()
                    nc.sync.dma_start(out=dbg[:], in_=t_[:])

            stP.close()

            # ---- phase B: cross MLP over pair columns ----
            with (
                tc.tile_pool(name="bout", bufs=3) as bop,
                tc.tile_pool(name="bh", bufs=4) as bhp,
                tc.tile_pool(name="psB", bufs=1, space="PSUM") as psB,
            ):
                for bb in range(NDSL // 2):
                    dint = dints[bb]
                    outt = bop.tile([D, 2 * N1], BF16, name="outt")
                    for s in range(4):       # 4 pairs per 2-dd block
                        u = 4 * bb + s       # pair index 0..31
                        sl = slice(1024 * s, 1024 * s + 1024)
                        pe_pair = u in PAIR_PE
                        hp2 = None
                        if u >= EARLY_P:
                            hp2 = psB.tile([D, 1024], F32, name="hp2",
                                           tag="hp", bufs=2)
                            for hh in range(2):
                                c = 2 * u + hh
                                hsl = slice(512 * hh, 512 * hh + 512)
                                dsl = slice(512 * (2 * s + hh),
                                            512 * (2 * s + hh) + 512)
                                nc.tensor.matmul(hp2[:, hsl], W["cwd"],
                                                 dint[:, dsl], start=True,
                                                 stop=not pe_pair)
                                if pe_pair:
                                    nc.tensor.matmul(hp2[:, hsl], W["cwk"],
                                                     KB[:], start=False,
                                                     stop=False)
                                    qb = DN1[:, 4 * c:4 * c + 4] \
                                        .unsqueeze(2) \
                                        .broadcast_to([D, 4, 128])
                                    nc.tensor.matmul(
                                        hp2[:, hsl].rearrange(
                                            "p (a b) -> p a b", a=4),
                                        W["cwq"], qb, start=False,
                                        stop=True)
                        hr2 = bhp.tile([D, 1024], BF16, name="hr2")
                        if pe_pair:
                            nc.scalar.activation(hr2[:], hp2[:], AF.Relu,
                                                 bias=CB1C)
                        else:
                            if u < EARLY_P:
                                hs2 = HS[u]
                            else:
                                hs2 = bhp.tile([D, 1024], BF16, name="hs2",
                                               tag="hs", bufs=3)
                                nc.vector.tensor_tensor(hs2[:], hp2[:],
                                                        CKB8[:],
                                                        op=ALU.add)
                            for g in range(8):
                                gs = slice(128 * g, 128 * g + 128)
                                qcol = CQB1[:, 8 * u + g:8 * u + g + 1]
                                if u in RELU_ACT:
                                    nc.scalar.activation(
                                        hr2[:, gs], hs2[:, gs], AF.Relu,
                                        bias=qcol)
                                elif u in RELU_DVE:
                                    nc.vector.tensor_scalar(
                                        hr2[:, gs], hs2[:, gs], qcol, 0.0,
                                        op0=ALU.add, op1=ALU.max)
                                else:
                                    nc.gpsimd.tensor_scalar(
                                        hr2[:, gs], hs2[:, gs], qcol, 0.0,
                                        op0=ALU.add, op1=ALU.max)
                        op2 = psB.tile([D, 1024], F32, name="op2",
                                       tag="op", bufs=2)
                        for hh in range(2):
                            hsl = slice(512 * hh, 512 * hh + 512)
                            nc.tensor.matmul(op2[:, hsl], W["cw2"],
                                             hr2[:, hsl], start=True,
                                             stop=True)
                        if u in OD_DVE:
                            nc.vector.tensor_scalar_add(outt[:, sl], op2,
                                                        CB2C)
                        else:
                            nc.scalar.activation(outt[:, sl], op2,
                                                 AF.Identity, bias=CB2C)
                    nc.sync.dma_start(
                        out=out[:, 2 * N1 * bb:2 * N1 * (bb + 1)],
                        in_=outt[:])

    nc.compile()
    return nc


def _host_prep(inputs):
    g = {k: np.asarray(v, dtype=np.float32) for k, v in inputs.items()}
    perm = np.empty(D, dtype=np.int64)
    for h in range(H):
        for d in range(DH):
            perm[DH * h + d] = H * d + h

    w1T = g["a_w1"].T
    w2T = g["a_w2"].T
    cw1T = g["c_w1"].T
    wqTp = g["a_wq"].T[:, perm] * SCALE
    blocks = {
        "wkT": g["a_wk"].T[:, perm],
        "wvT": g["a_wv"].T[:, perm],
        "wmT": g["a_wm"].T[perm, :],
        "w1T00": w1T[0:D, 0:D], "w1T10": w1T[D:2 * D, 0:D],
        "w1T01": w1T[0:D, D:2 * D], "w1T11": w1T[D:2 * D, D:2 * D],
        "w2T0": w2T[0:D, :], "w2T1": w2T[D:2 * D, :],
        "cwq": cw1T[0:D, :], "cwk": cw1T[D:2 * D, :],
        "cwd": cw1T[2 * D:3 * D, :], "cw2": g["c_w2"].T,
    }
    for h in range(H):
        m = np.zeros((D, D), dtype=np.float32)
        m[:, DH * h:DH * (h + 1)] = wqTp[:, DH * h:DH * (h + 1)]
        blocks[f"wqT{h}"] = m
    d0 = g["desc0"][0]
    d1 = g["desc1"][0]

    bigf = np.zeros((D, NBIAS), dtype=np.float32)
    bqp = g["a_bq"][perm] * SCALE
    for h in range(H):
        bigf[DH * h:DH * (h + 1), h] = bqp[DH * h:DH * (h + 1)]
    bigf[:, 4] = g["a_bk"][perm]
    bigf[:, 5] = g["a_bm"] + g["a_wm"] @ g["a_bv"]
    bigf[:, 6] = g["a_b1"][0:D]
    bigf[:, 7] = g["a_b1"][D:2 * D]
    bigf[:, 8] = g["a_b2"]
    bigf[:, 9] = g["c_b1"]
    bigf[:, 10] = g["c_b2"]

    bf = ml_dtypes.bfloat16
    dist = g["dist"][0].astype(bf)
    d1c = np.ascontiguousarray(d1.astype(bf))
    in_maps = []
    for p in range(NCORES):
        bigb = np.concatenate(
            [blocks[nm] for nm in _WNAMES]
            + [d0, d1[:, NQL * p:NQL * (p + 1)],
               np.ones((D, DH), dtype=np.float32)], axis=1).astype(bf)
        in_maps.append({
            "bigb": np.ascontiguousarray(bigb),
            "d1d": d1c,
            "bigf": bigf,
            "dist": np.ascontiguousarray(dist[NDSL * p:NDSL * (p + 1)]),
        })
    return in_maps


def kernel(**inputs):
    if "nc" not in _CACHE:
        _CACHE["nc"] = _build()
    nc = _CACHE["nc"]
    in_maps = _host_prep(inputs)
    res = run_bass_kernel_spmd(nc, in_maps, list(range(NCORES))).results
    full = np.concatenate(
        [res[p]["out"].astype(np.float32) for p in range(NCORES)], axis=1)
    return full.reshape(D, N1, N0)
